# revision 34
# baseline (speedup 1.0000x reference)
"""RelGraphConv (R-GCN layer + concat-MLP) Bass kernel for 8 trn2 NeuronCores.

Strategy (dst-node sharding, graph-parallel):
  - Core c owns nodes [c*12500, (c+1)*12500). It processes the edges whose dst
    falls in its slab and produces the output rows for its nodes.
  - x is replicated to every core in bf16 (gather source); the per-core x^T
    slab feeds the MLP in feature-major layout.
  - Per edge: gather x[src] in bf16 (indirect row gather), one-hot matmul
    (segment-sum into per-(window,relation) zT in PSUM), zT @ W_rel
    accumulated into AGG, then the fused concat-MLP:
        mid = tanh(x@Wx_eff + AGG@W1m + b1_eff);  out = [x, mid]@W2 + b2
    where Wx_eff = W1[:D] + loop_w@W1[D:], b1_eff = b1 + rel_bias@W1[D:]
    are folded on the host.
  - vs the first working version: everything bf16 on-chip (halves gather
    bytes), one-hot built per tile with DVE tensor_scalar (per-partition
    scalar) instead of broadcast tensor_tensor, one-hot matmuls write only
    the slot-subrange their (slot-sorted) edges cover, the last tile of each
    group gathers only the rows it needs, zt evacuation split ACT/DVE,
    gathers prefetched two superblocks ahead.
  - Measured: 1.26ms/iter, rel_err 3.6e-3. The critical path is the SWDGE
    (GpSimd Q7) descriptor emission of the 784 per-tile indirect gathers at
    ~1.26us busy + ~0.31us dispatch gap per call (~78% of the span);
    TensorE is ~31% busy, DVE ~26%, ACT ~9%, DMA engines ~19%. Multi-index
    batched gathers would fix this but are broken in HW (see
    trn2-indirect-dma-quirks memory / gather_diag.py): the DGE uses only
    idx[p,0] and fetches consecutive rows. int16 dma_gather requires
    chunking x below 32768 rows, which fragments the dst-sorted schedule
    and shifts the cost into one-hot builds (worse).
"""
import sys
import types

sys.path.insert(0, "/opt/trn_rl_repo")

import numpy as np

# problem shapes (hardcoded per contract)
N, E, D, OUT, R = 100000, 640000, 128, 128, 8
P = 8
NS = N // P            # 12500 nodes per core
WIN = 256              # one-hot window (PSUM free dim per relation)
NWIN = (NS + WIN - 1) // WIN   # 49 windows per core

SUPER = 4              # windows per super-block (gather prefetch unit)
NSUP = (NWIN + SUPER - 1) // SUPER  # 13


def _build_schedule(src, dst, etype):
    """Tiles keyed (w, r), edges slot-sorted within each group. Tile count and
    row counts are the max over cores so all cores share one program. Each
    tile records the static slot subrange [a, b] its one-hot matmul writes;
    ranges tile [0, WIN) per group (with overlaps at shared boundary slots)
    so every PSUM column is written."""
    src = np.asarray(src).astype(np.int64)
    dst = np.asarray(dst).astype(np.int64)
    etype = np.asarray(etype).astype(np.int64)

    core = dst // NS
    dl_all = dst - core * NS
    w_all = dl_all // WIN
    slot_in_win = dl_all - w_all * WIN

    NG = NWIN * R
    g_all = w_all * R + etype

    # per-core sort by (group, slot)
    starts = np.zeros((P, NG), dtype=np.int64)
    ends = np.zeros((P, NG), dtype=np.int64)
    src_s = [None] * P
    slot_s = [None] * P
    for c in range(P):
        m = core == c
        key = g_all[m] * WIN + slot_in_win[m]
        order = np.argsort(key, kind="stable")
        g_sorted = g_all[m][order]
        src_s[c] = src[m][order]
        slot_s[c] = slot_in_win[m][order]
        starts[c] = np.searchsorted(g_sorted, np.arange(NG))
        ends[c] = np.searchsorted(g_sorted, np.arange(NG) + 1)

    counts = ends - starts                       # [P, NG]
    max8 = counts.max(axis=0)                    # [NG]
    T_g = np.maximum(1, (max8 + 127) // 128)

    # build tiles in (sb, w, r, t) order with static ranges + row counts
    tiles = []  # dict per tile
    sb_ntiles = [0] * NSUP
    sb_ncols = [0] * NSUP   # gather buffer columns (128 per tile slot)
    for w in range(NWIN):
        sb = w // SUPER
        for r in range(R):
            g = w * R + r
            T = int(T_g[g])
            raw_a = np.full(T, WIN, dtype=np.int64)
            raw_b = np.zeros(T, dtype=np.int64)
            nrows = np.zeros(T, dtype=np.int64)
            for t in range(T):
                nr = int(min(128, max8[g] - 128 * t))
                nr = max(nr, 1)
                nrows[t] = nr
                for c in range(P):
                    lo = starts[c, g] + 128 * t
                    hi = min(starts[c, g] + 128 * (t + 1), ends[c, g])
                    if hi > lo:
                        raw_a[t] = min(raw_a[t], slot_s[c][lo])
                        raw_b[t] = max(raw_b[t], slot_s[c][hi - 1])
            # coverage fixups: a_0=0, b_last=WIN-1, fill gaps
            raw_a[0] = 0
            raw_b[T - 1] = WIN - 1
            for t in range(T - 1):
                raw_b[t] = max(raw_b[t], raw_a[t + 1] - 1)
                raw_b[t] = max(raw_b[t], raw_a[t])
            for t in range(T):
                tiles.append(
                    dict(sb=sb, w=w, r=r, t=t, g=g,
                         a=int(raw_a[t]), b=int(raw_b[t]),
                         nrows=int(nrows[t]),
                         ft_sb=sb_ntiles[sb], ft=len(tiles))
                )
                sb_ntiles[sb] += 1
                sb_ncols[sb] += 128
    n_ft_total = len(tiles)
    max_sb_tiles = max(sb_ntiles)

    idx_arrs = np.zeros((P, 128, n_ft_total), dtype=np.int32)
    slot_arrs = np.full((P, 128, n_ft_total), -1.0, dtype=np.float32)
    for c in range(P):
        for tl in tiles:
            g, t, ft = tl["g"], tl["t"], tl["ft"]
            lo = starts[c, g] + 128 * t
            hi = min(starts[c, g] + 128 * (t + 1), ends[c, g])
            nreal = max(0, int(hi - lo))
            if nreal > 0:
                idx_arrs[c, :nreal, ft] = src_s[c][lo:hi]
                slot_arrs[c, :nreal, ft] = slot_s[c][lo:hi]

    return (
        {
            "tiles": tiles,
            "n_ft_total": n_ft_total,
            "max_sb_tiles": max_sb_tiles,
            "sb_ntiles": sb_ntiles,
        },
        idx_arrs,
        slot_arrs,
    )


def _build_program(sched):
    import concourse.bass as bass
    import concourse.bacc as bacc
    import concourse.tile as tile
    from concourse import mybir

    F32 = mybir.dt.float32
    BF16 = mybir.dt.bfloat16
    AF = mybir.ActivationFunctionType

    tiles = sched["tiles"]
    n_ft_total = sched["n_ft_total"]
    max_sb_tiles = sched["max_sb_tiles"]

    nc = bacc.Bacc(None, target_bir_lowering=False)

    x_full = nc.dram_tensor("x_full", [N, D], BF16, kind="ExternalInput")
    xT_loc = nc.dram_tensor("xT_loc", [D, NWIN * WIN], BF16, kind="ExternalInput")
    idx_d = nc.dram_tensor("idx_d", [128, n_ft_total], mybir.dt.int32,
                           kind="ExternalInput")
    slot_d = nc.dram_tensor("slot_d", [128, n_ft_total], F32, kind="ExternalInput")
    iota_d = nc.dram_tensor("iota_d", [128, WIN], BF16, kind="ExternalInput")
    w_rel_d = nc.dram_tensor("w_rel_d", [D, R * OUT], BF16, kind="ExternalInput")
    wx_eff_d = nc.dram_tensor("wx_eff_d", [D, 256], BF16, kind="ExternalInput")
    w1m_d = nc.dram_tensor("w1m_d", [D, 256], BF16, kind="ExternalInput")
    w2_d = nc.dram_tensor("w2_d", [384, OUT], BF16, kind="ExternalInput")
    b1_d = nc.dram_tensor("b1_d", [128, 2], F32, kind="ExternalInput")
    b2_d = nc.dram_tensor("b2_d", [128, 1], F32, kind="ExternalInput")
    out_d = nc.dram_tensor("out_fm", [128, NWIN * WIN], BF16, kind="ExternalOutput")

    with tile.TileContext(nc) as tc:
        with (
            tc.tile_pool(name="const", bufs=1) as constp,
            tc.tile_pool(name="gbuf", bufs=1) as gbufp,
            tc.tile_pool(name="xfm", bufs=3) as xfmp,
            tc.tile_pool(name="pt", bufs=6) as ptp,
            tc.tile_pool(name="ztsb", bufs=3) as ztsbp,
            tc.tile_pool(name="aggsb", bufs=2) as aggsbp,
            tc.tile_pool(name="midsb", bufs=2) as midsbp,
            tc.tile_pool(name="outsb", bufs=2) as outsbp,
            tc.tile_pool(name="zt_ps", bufs=2, space="PSUM") as ztps,
            tc.tile_pool(name="agg_ps", bufs=2, space="PSUM") as aggps,
            tc.tile_pool(name="mid_ps", bufs=2, space="PSUM") as midps,
            tc.tile_pool(name="out_ps", bufs=2, space="PSUM") as outps,
        ):
            iota_t = constp.tile([128, WIN], BF16)
            nc.sync.dma_start(out=iota_t[:], in_=iota_d[:])
            w_rel_t = constp.tile([128, R * OUT], BF16)
            nc.sync.dma_start(out=w_rel_t[:], in_=w_rel_d[:])
            wx_eff_t = constp.tile([128, 256], BF16)
            nc.sync.dma_start(out=wx_eff_t[:], in_=wx_eff_d[:])
            w1m_t = constp.tile([128, 256], BF16)
            nc.sync.dma_start(out=w1m_t[:], in_=w1m_d[:])
            w2_t = constp.tile([128, 3 * OUT], BF16)
            for kblk in range(3):
                nc.sync.dma_start(
                    out=w2_t[:, kblk * OUT : (kblk + 1) * OUT],
                    in_=w2_d[kblk * 128 : (kblk + 1) * 128, :],
                )
            b1_t = constp.tile([128, 2], F32)
            nc.sync.dma_start(out=b1_t[:], in_=b1_d[:])
            b2_t = constp.tile([128, 1], F32)
            nc.sync.dma_start(out=b2_t[:], in_=b2_d[:])
            slot_t = constp.tile([128, n_ft_total], F32)
            nc.sync.dma_start(out=slot_t[:], in_=slot_d[:])
            idx_t = constp.tile([128, n_ft_total], mybir.dt.int32)
            nc.sync.dma_start(out=idx_t[:], in_=idx_d[:])

            gbuf = []
            for i in range(3):
                g_tile = gbufp.tile([128, max_sb_tiles * 128], BF16, tag=f"g{i}")
                gbuf.append(g_tile)

            tiles_by_sb = {}
            tiles_by_wr = {}
            for tl in tiles:
                tiles_by_sb.setdefault(tl["sb"], []).append(tl)
                tiles_by_wr.setdefault((tl["w"], tl["r"]), []).append(tl)

            def emit_gathers(sb):
                # per-tile indirect calls (multi-column offset APs are broken
                # on HW: the DGE uses only idx[p,0] and fetches k consecutive
                # rows; see gather_diag.py). ~1.26us SWDGE emission per call
                # is the kernel's critical path -> prefetch 2 superblocks
                # ahead so the Q7 never idles.
                buf = gbuf[sb % 3]
                for tl in tiles_by_sb[sb]:
                    nr, ft_sb, ft = tl["nrows"], tl["ft_sb"], tl["ft"]
                    nc.gpsimd.indirect_dma_start(
                        out=buf[:nr, ft_sb * 128 : ft_sb * 128 + 128],
                        out_offset=None,
                        in_=x_full[:],
                        in_offset=bass.IndirectOffsetOnAxis(
                            ap=idx_t[:nr, ft : ft + 1], axis=0
                        ),
                    )

            def emit_onehot(tl, zt_pair, half):
                """pt build (tensor_scalar, DVE/GpSimd alternating) +
                subrange one-hot matmul."""
                nr, a, b, ft = tl["nrows"], tl["a"], tl["b"], tl["ft"]
                width = b - a + 1
                buf = gbuf[tl["sb"] % 3]
                pt = ptp.tile([128, WIN], BF16, tag="pt")
                eng = nc.vector
                eng.tensor_scalar(
                    out=pt[:nr, :width],
                    in0=iota_t[:nr, a : a + width],
                    scalar1=slot_t[:nr, ft : ft + 1],
                    scalar2=None,
                    op0=mybir.AluOpType.is_equal,
                )
                grp = tiles_by_wr[(tl["w"], tl["r"])]
                nc.tensor.matmul(
                    out=zt_pair[:, half * WIN + a : half * WIN + a + width],
                    lhsT=buf[:nr, tl["ft_sb"] * 128 : tl["ft_sb"] * 128 + 128],
                    rhs=pt[:nr, :width],
                    start=(tl["t"] == 0),
                    stop=(tl["t"] == len(grp) - 1),
                )

            emit_gathers(0)
            emit_gathers(1)
            for sb in range(NSUP):
                if sb + 2 < NSUP:
                    emit_gathers(sb + 2)
                w_lo = sb * SUPER
                w_hi = min((sb + 1) * SUPER, NWIN)
                for w in range(w_lo, w_hi):
                    x_fm = xfmp.tile([128, WIN], BF16, tag="xfm")
                    nc.sync.dma_start(
                        out=x_fm[:], in_=xT_loc[:, w * WIN : (w + 1) * WIN]
                    )
                    agg = aggps.tile([128, WIN], F32, space="PSUM", tag="agg")
                    # software-pipeline one stage: onehot(p+1) is emitted
                    # before wrel(p) so PE doesn't stall on the ACT zt copy
                    zts = []
                    for rpair in range(R // 2):
                        zt_pair = ztps.tile([128, 2 * WIN], F32, space="PSUM",
                                            tag="zt")
                        for half_r in range(2):
                            r = rpair * 2 + half_r
                            for tl in tiles_by_wr[(w, r)]:
                                emit_onehot(tl, zt_pair, half_r)
                        zt_sb = ztsbp.tile([128, 2 * WIN], BF16, tag="ztsb")
                        if rpair % 2 == 0:
                            nc.scalar.activation(out=zt_sb[:], in_=zt_pair[:],
                                                 func=AF.Copy)
                        else:
                            nc.vector.tensor_copy(out=zt_sb[:], in_=zt_pair[:])
                        zts.append(zt_sb)
                    for rpair in range(R // 2):
                        zt_sb = zts[rpair]
                        for half_r in range(2):
                            r = rpair * 2 + half_r
                            nc.tensor.matmul(
                                out=agg[:],
                                lhsT=w_rel_t[:, r * OUT : (r + 1) * OUT],
                                rhs=zt_sb[:, half_r * WIN : (half_r + 1) * WIN],
                                start=(r == 0),
                                stop=(r == R - 1),
                            )
                    agg_sb = aggsbp.tile([128, WIN], BF16, tag="aggsb")
                    nc.vector.tensor_copy(out=agg_sb[:], in_=agg[:])

                    mid_pair = midps.tile([128, 2 * WIN], F32, space="PSUM",
                                          tag="mid")
                    for j in range(2):
                        nc.tensor.matmul(
                            out=mid_pair[:, j * WIN : (j + 1) * WIN],
                            lhsT=wx_eff_t[:, j * 128 : (j + 1) * 128],
                            rhs=x_fm[:], start=True, stop=False,
                        )
                        nc.tensor.matmul(
                            out=mid_pair[:, j * WIN : (j + 1) * WIN],
                            lhsT=w1m_t[:, j * 128 : (j + 1) * 128],
                            rhs=agg_sb[:], start=False, stop=True,
                        )
                    mid_sb = midsbp.tile([128, 2 * WIN], BF16, tag="midsb")
                    for j in range(2):
                        nc.scalar.activation(
                            out=mid_sb[:, j * WIN : (j + 1) * WIN],
                            in_=mid_pair[:, j * WIN : (j + 1) * WIN],
                            func=AF.Tanh, bias=b1_t[:, j : j + 1],
                        )
                    out_ps_t = outps.tile([128, WIN], F32, space="PSUM",
                                          tag="outps")
                    for kblk, rhs_t in ((0, x_fm[:]), (1, mid_sb[:, 0:WIN]),
                                        (2, mid_sb[:, WIN : 2 * WIN])):
                        nc.tensor.matmul(
                            out=out_ps_t[:],
                            lhsT=w2_t[:, kblk * OUT : (kblk + 1) * OUT],
                            rhs=rhs_t, start=(kblk == 0), stop=(kblk == 2),
                        )
                    out_sb = outsbp.tile([128, WIN], BF16, tag="outsb")
                    nc.vector.tensor_scalar(
                        out=out_sb[:], in0=out_ps_t[:],
                        scalar1=b2_t[:, 0:1], scalar2=None,
                        op0=mybir.AluOpType.add,
                    )
                    nc.sync.dma_start(
                        out=out_d[:, w * WIN : (w + 1) * WIN], in_=out_sb[:]
                    )

    nc.compile()
    return nc


def _install_ntff_hook():
    try:
        import antenv

        if "antenv.axon_hooks" in sys.modules:
            return
        mod = types.ModuleType("antenv.axon_hooks")
        _h = {"hook": None}
        mod.set_axon_ntff_profile_hook = lambda h: _h.update(hook=h)
        mod.get_axon_ntff_profile_hook = lambda: _h["hook"]
        sys.modules["antenv.axon_hooks"] = mod
        antenv.axon_hooks = mod
        from trn_agent_boot.trn_boot import _ntff_profile_via_ctypes

        mod.set_axon_ntff_profile_hook(
            _ntff_profile_via_ctypes("/opt/axon/libaxon_pjrt.so")
        )
    except Exception:
        pass


def _prepare(x, src, dst, etype, W_rel, loop_w, rel_bias, W1, b1, W2, b2):
    """Build (nc, in_maps) — shared by kernel() and the sim checker."""
    x = np.asarray(x, dtype=np.float32)
    W_rel = np.asarray(W_rel, dtype=np.float32)
    loop_w = np.asarray(loop_w, dtype=np.float32)
    rel_bias = np.asarray(rel_bias, dtype=np.float32)
    W1 = np.asarray(W1, dtype=np.float32)
    b1 = np.asarray(b1, dtype=np.float32)
    W2 = np.asarray(W2, dtype=np.float32)
    b2 = np.asarray(b2, dtype=np.float32)

    sched, idx_arrs, slot_arrs = _build_schedule(src, dst, etype)

    # host-folded weights
    W1x, W1m = W1[:D], W1[D:]
    wx_eff = W1x + loop_w @ W1m              # [128, 256]
    b1_eff = b1 + rel_bias @ W1m             # [256]
    w_rel_flat = np.concatenate([W_rel[r] for r in range(R)], axis=1)
    iota = np.tile(np.arange(WIN, dtype=np.float32), (128, 1))
    b1_cols = b1_eff.reshape(2, 128).T.copy()  # [128, 2]
    b2_col = b2.reshape(128, 1).copy()

    def bf16(a):
        import ml_dtypes

        return np.asarray(a, dtype=np.float32).astype(ml_dtypes.bfloat16)

    nc = _build_program(sched)

    x_bf = bf16(x)
    in_maps = []
    for c in range(P):
        xT = np.zeros((D, NWIN * WIN), dtype=np.float32)
        xT[:, :NS] = x[c * NS : (c + 1) * NS].T
        in_maps.append(
            {
                "x_full": x_bf,
                "xT_loc": bf16(xT),
                "idx_d": idx_arrs[c],
                "slot_d": slot_arrs[c],
                "iota_d": bf16(iota),
                "w_rel_d": bf16(w_rel_flat),
                "wx_eff_d": bf16(wx_eff),
                "w1m_d": bf16(W1m),
                "w2_d": bf16(W2),
                "b1_d": b1_cols,
                "b2_d": b2_col,
            }
        )
    return nc, in_maps


def kernel(x, src, dst, etype, W_rel, loop_w, rel_bias, W1, b1, W2, b2,
           trace=False):
    from concourse.bass_utils import run_bass_kernel_spmd

    _install_ntff_hook()

    nc, in_maps = _prepare(x, src, dst, etype, W_rel, loop_w, rel_bias,
                           W1, b1, W2, b2)

    res = run_bass_kernel_spmd(nc, in_maps, core_ids=list(range(P)), trace=trace)
    if trace:
        kernel.last_exec_time_ns = res.exec_time_ns
        kernel.last_results = res
        try:
            kernel.last_trace_path = (
                res.instructions_and_trace[1] if res.instructions_and_trace else None
            )
        except Exception:
            kernel.last_trace_path = None
        kernel.last_profile_json = getattr(res, "profile_json", None)
        try:
            import pickle

            insts = (
                res.instructions_and_trace[0] if res.instructions_and_trace else []
            )
            rows = [
                (i.engine, i.timestamp, i.duration, i.bir_instruction_name,
                 i.name, i.source_line)
                for i in insts
            ]
            with open("/tmp/last_insts.pkl", "wb") as f:
                pickle.dump(
                    {"rows": rows, "exec_time_ns": res.exec_time_ns,
                     "profile_json": kernel.last_profile_json}, f)
        except Exception:
            pass

    out = np.empty((N, OUT), dtype=np.float32)
    for c in range(P):
        out[c * NS : (c + 1) * NS] = (
            res.results[c]["out_fm"][:, :NS].astype(np.float32).T
        )
    return out


# revision 45
# speedup vs baseline: 1.0008x; 1.0008x over previous
"""RelGraphConv (R-GCN layer + concat-MLP) Bass kernel for 8 trn2 NeuronCores.

Strategy (dst-node sharding, graph-parallel):
  - Core c owns nodes [c*12500, (c+1)*12500). It processes the edges whose dst
    falls in its slab and produces the output rows for its nodes.
  - x is replicated to every core in bf16 (gather source); the per-core x^T
    slab feeds the MLP in feature-major layout.
  - Per edge: gather x[src] in bf16 (indirect row gather), one-hot matmul
    (segment-sum into per-(window,relation) zT in PSUM), zT @ W_rel
    accumulated into AGG, then the fused concat-MLP:
        mid = tanh(x@Wx_eff + AGG@W1m + b1_eff);  out = [x, mid]@W2 + b2
    where Wx_eff = W1[:D] + loop_w@W1[D:], b1_eff = b1 + rel_bias@W1[D:]
    are folded on the host.
  - vs the first working version: everything bf16 on-chip (halves gather
    bytes), one-hot built per tile with DVE tensor_scalar (per-partition
    scalar) instead of broadcast tensor_tensor, one-hot matmuls write only
    the slot-subrange their (slot-sorted) edges cover, the last tile of each
    group gathers only the rows it needs, zt evacuation split ACT/DVE,
    gathers prefetched two superblocks ahead.
  - Measured: 1.26ms/iter, rel_err 3.6e-3. The critical path is the SWDGE
    (GpSimd Q7) descriptor emission of the 784 per-tile indirect gathers at
    ~1.26us busy + ~0.31us dispatch gap per call (~78% of the span);
    TensorE is ~31% busy, DVE ~26%, ACT ~9%, DMA engines ~19%. Multi-index
    batched gathers would fix this but are broken in HW (see
    trn2-indirect-dma-quirks memory / gather_diag.py): the DGE uses only
    idx[p,0] and fetches consecutive rows. int16 dma_gather requires
    chunking x below 32768 rows, which fragments the dst-sorted schedule
    and shifts the cost into one-hot builds (worse).
"""
import sys
import types

sys.path.insert(0, "/opt/trn_rl_repo")

import numpy as np

# problem shapes (hardcoded per contract)
N, E, D, OUT, R = 100000, 640000, 128, 128, 8
P = 8
NS = N // P            # 12500 nodes per core
WIN = 256              # one-hot window (PSUM free dim per relation)
NWIN = (NS + WIN - 1) // WIN   # 49 windows per core

SUPER = 4              # windows per super-block (gather prefetch unit)
NSUP = (NWIN + SUPER - 1) // SUPER  # 13


def _build_schedule(src, dst, etype):
    """Tiles keyed (w, r), edges slot-sorted within each group. Tile count and
    row counts are the max over cores so all cores share one program. Each
    tile records the static slot subrange [a, b] its one-hot matmul writes;
    ranges tile [0, WIN) per group (with overlaps at shared boundary slots)
    so every PSUM column is written."""
    src = np.asarray(src).astype(np.int64)
    dst = np.asarray(dst).astype(np.int64)
    etype = np.asarray(etype).astype(np.int64)

    core = dst // NS
    dl_all = dst - core * NS
    w_all = dl_all // WIN
    slot_in_win = dl_all - w_all * WIN

    NG = NWIN * R
    g_all = w_all * R + etype

    # per-core sort by (group, slot)
    starts = np.zeros((P, NG), dtype=np.int64)
    ends = np.zeros((P, NG), dtype=np.int64)
    src_s = [None] * P
    slot_s = [None] * P
    for c in range(P):
        m = core == c
        key = g_all[m] * WIN + slot_in_win[m]
        order = np.argsort(key, kind="stable")
        g_sorted = g_all[m][order]
        src_s[c] = src[m][order]
        slot_s[c] = slot_in_win[m][order]
        starts[c] = np.searchsorted(g_sorted, np.arange(NG))
        ends[c] = np.searchsorted(g_sorted, np.arange(NG) + 1)

    counts = ends - starts                       # [P, NG]
    max8 = counts.max(axis=0)                    # [NG]
    T_g = np.maximum(1, (max8 + 127) // 128)

    # build tiles in (sb, w, r, t) order with static ranges + row counts
    tiles = []  # dict per tile
    sb_ntiles = [0] * NSUP
    sb_ncols = [0] * NSUP   # gather buffer columns (128 per tile slot)
    for w in range(NWIN):
        sb = w // SUPER
        for r in range(R):
            g = w * R + r
            T = int(T_g[g])
            raw_a = np.full(T, WIN, dtype=np.int64)
            raw_b = np.zeros(T, dtype=np.int64)
            nrows = np.zeros(T, dtype=np.int64)
            for t in range(T):
                nr = int(min(128, max8[g] - 128 * t))
                nr = max(nr, 1)
                nrows[t] = nr
                for c in range(P):
                    lo = starts[c, g] + 128 * t
                    hi = min(starts[c, g] + 128 * (t + 1), ends[c, g])
                    if hi > lo:
                        raw_a[t] = min(raw_a[t], slot_s[c][lo])
                        raw_b[t] = max(raw_b[t], slot_s[c][hi - 1])
            # coverage fixups: a_0=0, b_last=WIN-1, fill gaps
            raw_a[0] = 0
            raw_b[T - 1] = WIN - 1
            for t in range(T - 1):
                raw_b[t] = max(raw_b[t], raw_a[t + 1] - 1)
                raw_b[t] = max(raw_b[t], raw_a[t])
            for t in range(T):
                tiles.append(
                    dict(sb=sb, w=w, r=r, t=t, g=g,
                         a=int(raw_a[t]), b=int(raw_b[t]),
                         nrows=int(nrows[t]),
                         ft_sb=sb_ntiles[sb], ft=len(tiles))
                )
                sb_ntiles[sb] += 1
                sb_ncols[sb] += 128
    n_ft_total = len(tiles)
    max_sb_tiles = max(sb_ntiles)

    idx_arrs = np.zeros((P, 128, n_ft_total), dtype=np.int32)
    slot_arrs = np.full((P, 128, n_ft_total), -1.0, dtype=np.float32)
    for c in range(P):
        for tl in tiles:
            g, t, ft = tl["g"], tl["t"], tl["ft"]
            lo = starts[c, g] + 128 * t
            hi = min(starts[c, g] + 128 * (t + 1), ends[c, g])
            nreal = max(0, int(hi - lo))
            if nreal > 0:
                idx_arrs[c, :nreal, ft] = src_s[c][lo:hi]
                slot_arrs[c, :nreal, ft] = slot_s[c][lo:hi]

    return (
        {
            "tiles": tiles,
            "n_ft_total": n_ft_total,
            "max_sb_tiles": max_sb_tiles,
            "sb_ntiles": sb_ntiles,
        },
        idx_arrs,
        slot_arrs,
    )


def _build_program(sched):
    import concourse.bass as bass
    import concourse.bacc as bacc
    import concourse.tile as tile
    from concourse import mybir

    F32 = mybir.dt.float32
    BF16 = mybir.dt.bfloat16
    AF = mybir.ActivationFunctionType

    tiles = sched["tiles"]
    n_ft_total = sched["n_ft_total"]
    max_sb_tiles = sched["max_sb_tiles"]

    nc = bacc.Bacc(None, target_bir_lowering=False)

    x_full = nc.dram_tensor("x_full", [N, D], BF16, kind="ExternalInput")
    xT_loc = nc.dram_tensor("xT_loc", [D, NWIN * WIN], BF16, kind="ExternalInput")
    idx_d = nc.dram_tensor("idx_d", [128, n_ft_total], mybir.dt.int32,
                           kind="ExternalInput")
    slot_d = nc.dram_tensor("slot_d", [128, n_ft_total], F32, kind="ExternalInput")
    iota_d = nc.dram_tensor("iota_d", [128, WIN], BF16, kind="ExternalInput")
    w_rel_d = nc.dram_tensor("w_rel_d", [D, R * OUT], BF16, kind="ExternalInput")
    wx_eff_d = nc.dram_tensor("wx_eff_d", [D, 256], BF16, kind="ExternalInput")
    w1m_d = nc.dram_tensor("w1m_d", [D, 256], BF16, kind="ExternalInput")
    w2_d = nc.dram_tensor("w2_d", [384, OUT], BF16, kind="ExternalInput")
    b1_d = nc.dram_tensor("b1_d", [128, 2], F32, kind="ExternalInput")
    b2_d = nc.dram_tensor("b2_d", [128, 1], F32, kind="ExternalInput")
    out_d = nc.dram_tensor("out_fm", [128, NWIN * WIN], BF16, kind="ExternalOutput")

    with tile.TileContext(nc) as tc:
        with (
            tc.tile_pool(name="const", bufs=1) as constp,
            tc.tile_pool(name="gbuf", bufs=1) as gbufp,
            tc.tile_pool(name="xfm", bufs=3) as xfmp,
            tc.tile_pool(name="pt", bufs=6) as ptp,
            tc.tile_pool(name="ztsb", bufs=3) as ztsbp,
            tc.tile_pool(name="aggsb", bufs=2) as aggsbp,
            tc.tile_pool(name="midsb", bufs=2) as midsbp,
            tc.tile_pool(name="outsb", bufs=2) as outsbp,
            tc.tile_pool(name="zt_ps", bufs=2, space="PSUM") as ztps,
            tc.tile_pool(name="agg_ps", bufs=2, space="PSUM") as aggps,
            tc.tile_pool(name="mid_ps", bufs=2, space="PSUM") as midps,
            tc.tile_pool(name="out_ps", bufs=2, space="PSUM") as outps,
        ):
            iota_t = constp.tile([128, WIN], BF16)
            nc.sync.dma_start(out=iota_t[:], in_=iota_d[:])
            w_rel_t = constp.tile([128, R * OUT], BF16)
            nc.sync.dma_start(out=w_rel_t[:], in_=w_rel_d[:])
            wx_eff_t = constp.tile([128, 256], BF16)
            nc.sync.dma_start(out=wx_eff_t[:], in_=wx_eff_d[:])
            w1m_t = constp.tile([128, 256], BF16)
            nc.sync.dma_start(out=w1m_t[:], in_=w1m_d[:])
            w2_t = constp.tile([128, 3 * OUT], BF16)
            for kblk in range(3):
                nc.sync.dma_start(
                    out=w2_t[:, kblk * OUT : (kblk + 1) * OUT],
                    in_=w2_d[kblk * 128 : (kblk + 1) * 128, :],
                )
            b1_t = constp.tile([128, 2], F32)
            nc.sync.dma_start(out=b1_t[:], in_=b1_d[:])
            b2_t = constp.tile([128, 1], F32)
            nc.sync.dma_start(out=b2_t[:], in_=b2_d[:])
            slot_t = constp.tile([128, n_ft_total], F32)
            nc.sync.dma_start(out=slot_t[:], in_=slot_d[:])
            idx_t = constp.tile([128, n_ft_total], mybir.dt.int32)
            nc.sync.dma_start(out=idx_t[:], in_=idx_d[:])

            gbuf = []
            for i in range(3):
                g_tile = gbufp.tile([128, max_sb_tiles * 128], BF16, tag=f"g{i}")
                gbuf.append(g_tile)

            tiles_by_sb = {}
            tiles_by_wr = {}
            for tl in tiles:
                tiles_by_sb.setdefault(tl["sb"], []).append(tl)
                tiles_by_wr.setdefault((tl["w"], tl["r"]), []).append(tl)

            def emit_gathers(sb):
                # per-tile indirect calls (multi-column offset APs are broken
                # on HW: the DGE uses only idx[p,0] and fetches k consecutive
                # rows; see gather_diag.py). ~1.26us SWDGE emission per call
                # is the kernel's critical path -> prefetch 2 superblocks
                # ahead so the Q7 never idles.
                buf = gbuf[sb % 3]
                for tl in tiles_by_sb[sb]:
                    nr, ft_sb, ft = tl["nrows"], tl["ft_sb"], tl["ft"]
                    nc.gpsimd.indirect_dma_start(
                        out=buf[:nr, ft_sb * 128 : ft_sb * 128 + 128],
                        out_offset=None,
                        in_=x_full[:],
                        in_offset=bass.IndirectOffsetOnAxis(
                            ap=idx_t[:nr, ft : ft + 1], axis=0
                        ),
                    )

            def emit_onehot(tl, zt_pair, half):
                """pt build (tensor_scalar, DVE/GpSimd alternating) +
                subrange one-hot matmul."""
                nr, a, b, ft = tl["nrows"], tl["a"], tl["b"], tl["ft"]
                width = b - a + 1
                buf = gbuf[tl["sb"] % 3]
                pt = ptp.tile([128, WIN], BF16, tag="pt")
                eng = nc.vector
                eng.tensor_scalar(
                    out=pt[:nr, :width],
                    in0=iota_t[:nr, a : a + width],
                    scalar1=slot_t[:nr, ft : ft + 1],
                    scalar2=None,
                    op0=mybir.AluOpType.is_equal,
                )
                grp = tiles_by_wr[(tl["w"], tl["r"])]
                nc.tensor.matmul(
                    out=zt_pair[:, half * WIN + a : half * WIN + a + width],
                    lhsT=buf[:nr, tl["ft_sb"] * 128 : tl["ft_sb"] * 128 + 128],
                    rhs=pt[:nr, :width],
                    start=(tl["t"] == 0),
                    stop=(tl["t"] == len(grp) - 1),
                )

            emit_gathers(0)
            emit_gathers(1)
            for sb in range(NSUP):
                if sb + 2 < NSUP:
                    emit_gathers(sb + 2)
                w_lo = sb * SUPER
                w_hi = min((sb + 1) * SUPER, NWIN)
                for w in range(w_lo, w_hi):
                    x_fm = xfmp.tile([128, WIN], BF16, tag="xfm")
                    nc.sync.dma_start(
                        out=x_fm[:], in_=xT_loc[:, w * WIN : (w + 1) * WIN]
                    )
                    agg = aggps.tile([128, WIN], F32, space="PSUM", tag="agg")
                    # software-pipeline one stage: onehot(p+1) is emitted
                    # before wrel(p) so PE doesn't stall on the ACT zt copy
                    zts = []
                    for rpair in range(R // 2):
                        zt_pair = ztps.tile([128, 2 * WIN], F32, space="PSUM",
                                            tag="zt")
                        for half_r in range(2):
                            r = rpair * 2 + half_r
                            for tl in tiles_by_wr[(w, r)]:
                                emit_onehot(tl, zt_pair, half_r)
                        zt_sb = ztsbp.tile([128, 2 * WIN], BF16, tag="ztsb")
                        if rpair % 2 == 0:
                            nc.scalar.activation(out=zt_sb[:], in_=zt_pair[:],
                                                 func=AF.Copy)
                        else:
                            nc.vector.tensor_copy(out=zt_sb[:], in_=zt_pair[:])
                        zts.append(zt_sb)
                    for rpair in range(R // 2):
                        zt_sb = zts[rpair]
                        for half_r in range(2):
                            r = rpair * 2 + half_r
                            nc.tensor.matmul(
                                out=agg[:],
                                lhsT=w_rel_t[:, r * OUT : (r + 1) * OUT],
                                rhs=zt_sb[:, half_r * WIN : (half_r + 1) * WIN],
                                start=(r == 0),
                                stop=(r == R - 1),
                            )
                    agg_sb = aggsbp.tile([128, WIN], BF16, tag="aggsb")
                    nc.vector.tensor_copy(out=agg_sb[:], in_=agg[:])

                    mid_pair = midps.tile([128, 2 * WIN], F32, space="PSUM",
                                          tag="mid")
                    for j in range(2):
                        nc.tensor.matmul(
                            out=mid_pair[:, j * WIN : (j + 1) * WIN],
                            lhsT=wx_eff_t[:, j * 128 : (j + 1) * 128],
                            rhs=x_fm[:], start=True, stop=False,
                        )
                        nc.tensor.matmul(
                            out=mid_pair[:, j * WIN : (j + 1) * WIN],
                            lhsT=w1m_t[:, j * 128 : (j + 1) * 128],
                            rhs=agg_sb[:], start=False, stop=True,
                        )
                    mid_sb = midsbp.tile([128, 2 * WIN], BF16, tag="midsb")
                    for j in range(2):
                        nc.scalar.activation(
                            out=mid_sb[:, j * WIN : (j + 1) * WIN],
                            in_=mid_pair[:, j * WIN : (j + 1) * WIN],
                            func=AF.Tanh, bias=b1_t[:, j : j + 1],
                        )
                    out_ps_t = outps.tile([128, WIN], F32, space="PSUM",
                                          tag="outps")
                    for kblk, rhs_t in ((0, x_fm[:]), (1, mid_sb[:, 0:WIN]),
                                        (2, mid_sb[:, WIN : 2 * WIN])):
                        nc.tensor.matmul(
                            out=out_ps_t[:],
                            lhsT=w2_t[:, kblk * OUT : (kblk + 1) * OUT],
                            rhs=rhs_t, start=(kblk == 0), stop=(kblk == 2),
                        )
                    out_sb = outsbp.tile([128, WIN], BF16, tag="outsb")
                    nc.vector.tensor_scalar(
                        out=out_sb[:], in0=out_ps_t[:],
                        scalar1=b2_t[:, 0:1], scalar2=None,
                        op0=mybir.AluOpType.add,
                    )
                    nc.sync.dma_start(
                        out=out_d[:, w * WIN : (w + 1) * WIN], in_=out_sb[:]
                    )

    nc.compile()
    return nc


def _install_ntff_hook():
    try:
        import antenv

        if "antenv.axon_hooks" in sys.modules:
            return
        mod = types.ModuleType("antenv.axon_hooks")
        _h = {"hook": None}
        mod.set_axon_ntff_profile_hook = lambda h: _h.update(hook=h)
        mod.get_axon_ntff_profile_hook = lambda: _h["hook"]
        sys.modules["antenv.axon_hooks"] = mod
        antenv.axon_hooks = mod
        from trn_agent_boot.trn_boot import _ntff_profile_via_ctypes

        mod.set_axon_ntff_profile_hook(
            _ntff_profile_via_ctypes("/opt/axon/libaxon_pjrt.so")
        )
    except Exception:
        pass


def _prepare(x, src, dst, etype, W_rel, loop_w, rel_bias, W1, b1, W2, b2):
    """Build (nc, in_maps) — shared by kernel() and the sim checker."""
    x = np.asarray(x, dtype=np.float32)
    W_rel = np.asarray(W_rel, dtype=np.float32)
    loop_w = np.asarray(loop_w, dtype=np.float32)
    rel_bias = np.asarray(rel_bias, dtype=np.float32)
    W1 = np.asarray(W1, dtype=np.float32)
    b1 = np.asarray(b1, dtype=np.float32)
    W2 = np.asarray(W2, dtype=np.float32)
    b2 = np.asarray(b2, dtype=np.float32)

    sched, idx_arrs, slot_arrs = _build_schedule(src, dst, etype)

    # host-folded weights
    W1x, W1m = W1[:D], W1[D:]
    wx_eff = W1x + loop_w @ W1m              # [128, 256]
    b1_eff = b1 + rel_bias @ W1m             # [256]
    w_rel_flat = np.concatenate([W_rel[r] for r in range(R)], axis=1)
    iota = np.tile(np.arange(WIN, dtype=np.float32), (128, 1))
    b1_cols = b1_eff.reshape(2, 128).T.copy()  # [128, 2]
    b2_col = b2.reshape(128, 1).copy()

    def bf16(a):
        import ml_dtypes

        return np.asarray(a, dtype=np.float32).astype(ml_dtypes.bfloat16)

    nc = _build_program(sched)

    x_bf = bf16(x)
    in_maps = []
    for c in range(P):
        xT = np.zeros((D, NWIN * WIN), dtype=np.float32)
        xT[:, :NS] = x[c * NS : (c + 1) * NS].T
        in_maps.append(
            {
                "x_full": x_bf,
                "xT_loc": bf16(xT),
                "idx_d": idx_arrs[c],
                "slot_d": slot_arrs[c],
                "iota_d": bf16(iota),
                "w_rel_d": bf16(w_rel_flat),
                "wx_eff_d": bf16(wx_eff),
                "w1m_d": bf16(W1m),
                "w2_d": bf16(W2),
                "b1_d": b1_cols,
                "b2_d": b2_col,
            }
        )
    return nc, in_maps


def kernel(x, src, dst, etype, W_rel, loop_w, rel_bias, W1, b1, W2, b2,
           trace=False):
    from concourse.bass_utils import run_bass_kernel_spmd

    _install_ntff_hook()

    nc, in_maps = _prepare(x, src, dst, etype, W_rel, loop_w, rel_bias,
                           W1, b1, W2, b2)

    res = run_bass_kernel_spmd(nc, in_maps, core_ids=list(range(P)), trace=trace)
    if trace:
        kernel.last_exec_time_ns = res.exec_time_ns
        kernel.last_results = res
        try:
            kernel.last_trace_path = (
                res.instructions_and_trace[1] if res.instructions_and_trace else None
            )
        except Exception:
            kernel.last_trace_path = None
        kernel.last_profile_json = getattr(res, "profile_json", None)
        try:
            import pickle

            insts = (
                res.instructions_and_trace[0] if res.instructions_and_trace else []
            )
            rows = [
                (i.engine, i.timestamp, i.duration, i.bir_instruction_name,
                 i.name, i.source_line)
                for i in insts
            ]
            with open("/tmp/last_insts.pkl", "wb") as f:
                pickle.dump(
                    {"rows": rows, "exec_time_ns": res.exec_time_ns,
                     "profile_json": kernel.last_profile_json}, f)
        except Exception:
            pass

    out = np.empty((N, OUT), dtype=np.float32)
    for c in range(P):
        out[c * NS : (c + 1) * NS] = (
            res.results[c]["out_fm"][:, :NS].astype(np.float32).T
        )
    return out
